# revision 2
# baseline (speedup 1.0000x reference)
"""Trainium2 Bass kernel for nn_DRA_C_65644280152592 (dense_transformer).

Data-parallel over batch B=8 (one sample per core). fp16 matmul operands
(fp32 PSUM), fp16 output (host casts back to fp32).

Key differences vs v1:
  - wq folded into the patch-embed weights on host (qw = pe_w_flat @ wq),
    so stage 1 directly produces q[s,196] with full-M (128) stationary
    tiles: 1024 matmuls of N=196 instead of 512 of N=512 (-24% PE).
  - wo folded into rc' (wrc = wo @ rcT): one fewer attention matmul chain.
  - q transposed on PE (8 tiny transposes) to feed sim = qT^T @ km.
  - InstanceNorm stats: partition-reduce and broadcast via tiny PE
    matmuls with ones vectors — no DRAM round-trip.
  - sim kept in PSUM through stats+softmax (no fp32 SBUF copy).
  - Stage-3 (mask conv) groups interleaved into the stats/softmax window
    to keep PE busy during the serial stats chain.
  - DMAs: two HWDGE rings (SP: qw stream + out; ACT: dec/consts), all
    transfers coalesced to >=512B elements, ~50 DMAs total vs 434.
"""
import sys

sys.path.insert(0, "/opt/trn_rl_repo")

import numpy as np

import concourse.bass as bass
import concourse.mybir as mybir
import concourse.tile as tile
from concourse import bacc
from concourse.bass_utils import run_bass_kernel_spmd

F16 = mybir.dt.float16
F32 = mybir.dt.float32
F8E3 = mybir.dt.float8e3
AF = mybir.ActivationFunctionType
AX = mybir.AxisListType

CIN, IMG, P = 512, 112, 8
NPR = 14
NPAT = NPR * NPR          # 196
DEC = SKIP = 512
EMB = 768
BN_EPS = 1e-3
IN_EPS = 1e-3
N_CORES = 8
SIM_N = float(SKIP * SKIP)

# f16 const blob column layout (per 128-partition row)
TRT_O = 0                  # trT:   6 * 196 = 1176
WK_O = TRT_O + 6 * NPAT    # wk:    6 * 512 = 3072
WV_O = WK_O + 6 * SKIP     # wv:    6 * 512 = 3072
MCT_O = WV_O + 6 * SKIP    # mcT:   4 * 512 = 2048
WRC_O = MCT_O + 4 * SKIP   # wrc:   4 * 512 = 2048
ID_O = WRC_O + 4 * SKIP    # ident: 128
C16_W = ID_O + 128

# f32 const blob: b1(4), b2(4), qb(4), psi(2)
C32_W = 16

W3 = 2 * NPAT              # 392 pixels per block (2 patch-rows)
NG = 3                     # blocks per stage-3 group (one out-DMA each)
NE = 8                     # early (maskE) blocks
NGRP = (32 - NE) // NG + 2 # maskE covers groups 0,1; stgp groups 2..9


def build_nc(repeat: int = 1):
    nc = bacc.Bacc(None)

    dec_d = nc.declare_dram_parameter("dec", [CIN, 64 * NPAT], F16, isOutput=False)
    qw_d = nc.declare_dram_parameter("qw", [16, 128, 16 * DEC], F8E3, isOutput=False)
    QW_SCALE = 32.0
    c16_d = nc.declare_dram_parameter("c16", [128, C16_W], F16, isOutput=False)
    c32_d = nc.declare_dram_parameter("c32", [128, C32_W], F32, isOutput=False)
    out_d = nc.declare_dram_parameter("out", [SKIP, 64 * NPAT], F16, isOutput=True)

    with tile.TileContext(nc) as tc:
        with tc.tile_pool(name="wts", bufs=1) as wts, \
             tc.tile_pool(name="qwp", bufs=2) as qwp, \
             tc.tile_pool(name="stg", bufs=2) as stgp, \
             tc.tile_pool(name="work", bufs=2) as work, \
             tc.tile_pool(name="acc", bufs=1, space="PSUM") as acc, \
             tc.tile_pool(name="ps", bufs=4, space="PSUM") as ps:

            def body():
                # ---------------- loads ----------------
                # SP ring: trT/wk/wv first (unblocks PE warmup), then the qw
                # stream. ACT ring: dec chunks (halved so stage 1 can start
                # after the first half), then the remaining consts.
                c16 = wts.tile([128, C16_W], F16, tag="c16")
                nc.sync.dma_start(out=c16[:, 0:WV_O], in_=c16_d[:, 0:WV_O])
                nc.sync.dma_start(out=c16[:, WV_O:WRC_O], in_=c16_d[:, WV_O:WRC_O])

                c32 = wts.tile([128, C32_W], F32, tag="c32")
                nc.scalar.dma_start(out=c32, in_=c32_d[:])

                HF = 32 * NPAT
                dec_sb = []
                for cb in range(4):
                    t = wts.tile([128, 64 * NPAT], F16, tag=f"dec{cb}",
                                 name=f"dec{cb}")
                    dec_sb.append(t)
                    for hh in range(2):
                        nc.scalar.dma_start(
                            out=t[:, hh * HF:(hh + 1) * HF],
                            in_=dec_d[cb * 128:(cb + 1) * 128,
                                      hh * HF:(hh + 1) * HF])
                nc.scalar.dma_start(out=c16[:, WRC_O:], in_=c16_d[:, WRC_O:])

                def c16v(off, k, w):
                    v = c16[:, off:off + k * w]
                    return v.rearrange("p (a b) -> p a b", b=w)
                trT = c16v(TRT_O, 6, NPAT)    # [128, 6, 196]
                wk = c16v(WK_O, 6, SKIP)      # [128, 6, 512]
                wv = c16v(WV_O, 6, SKIP)      # [128, 6, 512]
                mcT = c16v(MCT_O, 4, SKIP)    # [128, 4, 512]
                wrc = c16v(WRC_O, 4, SKIP)    # [128, 4, 512]
                ident = c16[:, ID_O:ID_O + 128]
                b1 = c32[:, 0:4]
                b2 = c32[:, 4:8]
                qb = c32[:, 8:12]
                psi = c32[0:1, 12:14]

                ones32c = wts.tile([128, 1], F32, tag="ones32c")
                nc.vector.memset(ones32c, 1.0)
                ones32r = wts.tile([1, 128], F32, tag="ones32r")
                nc.vector.memset(ones32r, 1.0)
                epsT = wts.tile([1, 1], F32, tag="epsT")
                nc.vector.memset(epsT, IN_EPS)

                # ---------------- early attention (PE warmup) ----------------
                # km[n,s] = sum_e trans[n,e] wk[e,s]; two 98-halves
                km16 = [wts.tile([98, SKIP], F16, tag=f"km{h}", name=f"km{h}")
                        for h in range(2)]
                for h in range(2):
                    pk = ps.tile([98, SKIP], F32, tag="pt")
                    for kt in range(6):
                        nc.tensor.matmul(pk, trT[:, kt, h * 98:(h + 1) * 98],
                                         wk[:, kt, :],
                                         start=(kt == 0), stop=(kt == 5))
                    nc.scalar.copy(km16[h], pk)

                # vT[t,n] = sum_e wv[e,t] trans[n,e]
                vT = wts.tile([128, 4, NPAT], F16, tag="vT")
                for m in range(4):
                    pv = ps.tile([128, NPAT], F32, tag="pt")
                    for kt in range(6):
                        nc.tensor.matmul(pv, wv[:, kt, m * 128:(m + 1) * 128],
                                         trT[:, kt, :],
                                         start=(kt == 0), stop=(kt == 5))
                    nc.scalar.copy(vT[:, m, :], pv)

                # ---------------- stage-3 helpers ----------------
                # maskE: first 8 blocks, computed early as PE filler during
                # the stats/softmax serial chain. stgp: remaining groups.
                maskE = wts.tile([128, 4, NE, W3], F16, tag="maskE")
                stg_tiles = {}

                def emit_mask_block(tgt, jb, blk):
                    p0 = blk * W3
                    for m in range(4):
                        pM = ps.tile([128, W3], F32, tag="pt")
                        for kt in range(4):
                            nc.tensor.matmul(
                                pM, mcT[:, kt, m * 128:(m + 1) * 128],
                                dec_sb[kt][:, p0:p0 + W3],
                                start=(kt == 0), stop=(kt == 3))
                        nc.scalar.activation(tgt[:, m, jb, :], pM, AF.Relu,
                                             bias=b1[:, m:m + 1])

                def emit_mask_group(g):
                    stg = stgp.tile([128, 4, NG, W3], F16, tag="stg",
                                    name=f"stg{g}")
                    stg_tiles[g] = stg
                    for jb in range(NG):
                        emit_mask_block(stg, jb, NE + (g - 2) * NG + jb)

                ov = out_d.rearrange("(m p) x -> p m x", m=4)

                def emit_mul_store(tile_, jb0, blk0, nblk, FIN, eng=None):
                    eng = eng or nc.vector
                    for m in range(4):
                        fbase = FIN[:, m, :]
                        fb = bass.AP(tensor=fbase.tensor, offset=fbase.offset,
                                     ap=[fbase.ap[0], [0, 2 * nblk], fbase.ap[1]])
                        v = tile_[:, m, jb0:jb0 + nblk, :] \
                            .rearrange("p a (c b) -> p (a c) b", b=NPAT)
                        eng.tensor_mul(v, v, fb)
                        nc.sync.dma_start(
                            out=ov[:, m:m + 1, blk0 * W3:(blk0 + nblk) * W3],
                            in_=tile_[:, m:m + 1, jb0:jb0 + nblk, :]
                            .rearrange("p m a b -> p m (a b)"))

                def emit_mul_store_group(g, FIN):
                    stg = stg_tiles.pop(g)
                    emit_mul_store(stg, 0, NE + (g - 2) * NG, NG, FIN)

                # ---------------- stage 1: q[s,n] = wq^T(pe(dec)+pe_b) --------
                pq = [acc.tile([128, SKIP], F32, tag=f"a{m}", name=f"pq{m}")
                      for m in range(4)]
                filler_at = {7: 0, 9: 1, 11: 2, 13: 3, 15: 4}
                for g8 in range(16):
                    qwt = qwp.tile([128, 16 * DEC], F8E3, tag="qwt")
                    nc.sync.dma_start(out=qwt, in_=qw_d[g8, :, :])
                    qwv = qwt.rearrange("p (a b) -> p a b", b=DEC)
                    for j in range(16):
                        k = g8 * 16 + j
                        cb, pp = k // 64, k % 64
                        xs = dec_sb[cb][:, pp * NPAT:(pp + 1) * NPAT]
                        for m in range(4):
                            nc.tensor.matmul(pq[m][:, 0:NPAT],
                                             qwv[:, j, m * 128:(m + 1) * 128],
                                             xs,
                                             start=(k == 0), stop=(k == 255))
                    if g8 in filler_at:
                        # absorb the qw DMA wait with a mask-conv block
                        emit_mask_block(maskE, filler_at[g8], filler_at[g8])
                emit_mask_block(maskE, 5, 5)
                # q16[s,n] = pq + qb (bias via activation), fp16
                q16 = wts.tile([128, 4, NPAT], F16, tag="q16")
                for m in range(4):
                    nc.scalar.activation(q16[:, m, :], pq[m][:, 0:NPAT],
                                         AF.Identity, scale=1.0 / QW_SCALE,
                                         bias=qb[:, m:m + 1])

                # ---------------- qT via PE transpose ----------------
                pqT = [ps.tile([98, 4 * 128], F16, tag="pt", name=f"pqT{h}")
                       for h in range(2)]
                for h in range(2):
                    for m in range(4):
                        nc.tensor.transpose(
                            pqT[h][:, m * 128:(m + 1) * 128],
                            q16[:, m, h * 98:(h + 1) * 98], ident)
                qT = [wts.tile([98, SKIP], F16, tag=f"qT{h}", name=f"qT{h}")
                      for h in range(2)]
                for h in range(2):
                    nc.scalar.copy(qT[h], pqT[h])
                emit_mask_block(maskE, 6, 6)

                # ---------------- sim = qT^T @ km  (PSUM resident) -------------
                psim = [acc.tile([128, SKIP], F32, tag=f"a{m}", name=f"psim{m}")
                        for m in range(4)]
                for m in range(4):
                    for h in range(2):
                        nc.tensor.matmul(psim[m],
                                         qT[h][:, m * 128:(m + 1) * 128],
                                         km16[h], start=(h == 0), stop=(h == 1))

                # early rmax: only needs sim, runs on DVE during fillers
                rmaxs = wts.tile([128, 4], F32, tag="rmaxs")
                for m in range(4):
                    nc.vector.reduce_max(rmaxs[:, m:m + 1], psim[m], axis=AX.X)

                # group-2 blocks 0,1 fill PE while the stats reduces run
                stg2 = stgp.tile([128, 4, NG, W3], F16, tag="stg", name="stg2")
                stg_tiles[2] = stg2
                emit_mask_block(stg2, 0, NE + 0)
                emit_mask_block(stg2, 1, NE + 1)


                # ---------------- instance-norm stats ----------------
                sm16 = wts.tile([128, 4, SKIP], F16, tag="sm16")
                stat8 = wts.tile([128, 8], F32, tag="stat8")
                for m in range(4):
                    nc.vector.reduce_sum(stat8[:, m:m + 1], psim[m], axis=AX.X)
                    # squares dumped into sm16 (overwritten by exp later)
                    nc.scalar.activation(sm16[:, m, :], psim[m], AF.Square,
                                         accum_out=stat8[:, 4 + m:5 + m])
                srow = wts.tile([128, 2], F32, tag="srow")
                nc.vector.reduce_sum(srow[:, 0:1], stat8[:, 0:4], axis=AX.X)
                nc.vector.reduce_sum(srow[:, 1:2], stat8[:, 4:8], axis=AX.X)
                # partition reduce on PE: [1,2] = ones[128,1].T @ srow[128,2]
                pst = ps.tile([1, 2], F32, tag="pt")
                nc.tensor.matmul(pst, ones32c, srow, start=True, stop=True)
                emit_mask_block(stg2, 2, NE + 2)
                sc = wts.tile([1, 8], F32, tag="sc")
                nc.scalar.copy(sc[:, 0:2], pst)
                # cols: 0=s,1=q,2=mu,3=ex2,4=musq,5=var,6=sqrt,7=rsig
                nc.scalar.mul(sc[:, 2:3], sc[:, 0:1], 1.0 / SIM_N)
                nc.scalar.mul(sc[:, 3:4], sc[:, 1:2], 1.0 / SIM_N)
                nc.vector.tensor_mul(sc[:, 4:5], sc[:, 2:3], sc[:, 2:3])
                nc.vector.tensor_sub(sc[:, 5:6], sc[:, 3:4], sc[:, 4:5])
                nc.scalar.activation(sc[:, 6:7], sc[:, 5:6], AF.Sqrt, bias=epsT)
                nc.vector.reciprocal(sc[:, 7:8], sc[:, 6:7])
                scal2 = wts.tile([1, 2], F32, tag="scal2")
                nc.vector.tensor_mul(scal2[:, 0:1], sc[:, 7:8], psi[:, 0:1])
                nc.scalar.mul(scal2[:, 1:2], scal2[:, 0:1], -1.0)
                # broadcast to 128 partitions on PE: ones[1,128].T @ scal2[1,2]
                pbc = ps.tile([128, 2], F32, tag="pt")
                nc.tensor.matmul(pbc, ones32r, scal2, start=True, stop=True)
                bc = wts.tile([128, 2], F32, tag="bc")
                nc.scalar.copy(bc, pbc)
                emit_mask_block(maskE, 7, 7)


                # ---------------- softmax (over free dim t) ----------------
                for m in range(4):
                    nm = work.tile([128, 1], F32, tag="nm")
                    nc.vector.tensor_mul(nm, rmaxs[:, m:m + 1], bc[:, 1:2])
                    rsum = work.tile([128, 1], F32, tag="rsum")
                    nc.scalar.activation(sm16[:, m, :], psim[m], AF.Exp,
                                         bias=nm, scale=bc[:, 0:1],
                                         accum_out=rsum)
                    rinv = work.tile([128, 1], F32, tag="rinv")
                    nc.vector.reciprocal(rinv, rsum)
                    nc.vector.tensor_scalar_mul(sm16[:, m, :], sm16[:, m, :],
                                                rinv)

                # ---------------- G2[t,c] = sum_s sm[s,t] wrc[s,c] -------------
                G2 = wts.tile([128, 4, SKIP], F16, tag="G2")
                for m in range(4):
                    pg = ps.tile([128, SKIP], F32, tag="pt")
                    for kt in range(4):
                        nc.tensor.matmul(pg, sm16[:, kt, m * 128:(m + 1) * 128],
                                         wrc[:, kt, :],
                                         start=(kt == 0), stop=(kt == 3))
                    nc.scalar.copy(G2[:, m, :], pg)

                # ---------------- FIN[c,n] = relu(G2^T @ vT + b2) --------------
                FIN = wts.tile([128, 4, NPAT], F16, tag="FIN")
                for m in range(4):
                    pf = ps.tile([128, NPAT], F32, tag="pt")
                    for kt in range(4):
                        nc.tensor.matmul(pf, G2[:, kt, m * 128:(m + 1) * 128],
                                         vT[:, kt, :],
                                         start=(kt == 0), stop=(kt == 3))
                    nc.scalar.activation(FIN[:, m, :], pf, AF.Relu,
                                         bias=b2[:, m:m + 1])

                # ---------------- stage 3 main ----------------
                # maskE covers blocks 0..7 (two out-groups of 4); stgp the rest.
                emit_mask_group(3)
                emit_mul_store(maskE, 0, 0, 4, FIN, eng=nc.gpsimd)
                emit_mul_store(maskE, 4, 4, 4, FIN, eng=nc.gpsimd)
                for g in range(4, NGRP):
                    emit_mul_store_group(g - 2, FIN)
                    emit_mask_group(g)
                emit_mul_store_group(NGRP - 2, FIN)
                emit_mul_store_group(NGRP - 1, FIN)

            if repeat == 1:
                body()
            else:
                with tc.For_i(0, repeat, 1):
                    body()
    nc.finalize()
    return nc


def prepare_in_maps(inputs: dict) -> list[dict]:
    f16 = np.float16
    decoder = np.asarray(inputs["decoder"], np.float32)
    trans = np.asarray(inputs["trans"], np.float32)
    pe_w = np.asarray(inputs["pe_w"], np.float32)
    pe_b = np.asarray(inputs["pe_b"], np.float32)
    mc_w = np.asarray(inputs["mc_w"], np.float32)
    mc_b = np.asarray(inputs["mc_b"], np.float32)
    bn1_g = np.asarray(inputs["bn1_g"], np.float32)
    bn1_b = np.asarray(inputs["bn1_b"], np.float32)
    bn1_m = np.asarray(inputs["bn1_m"], np.float32)
    bn1_v = np.asarray(inputs["bn1_v"], np.float32)
    wq = np.asarray(inputs["wq"], np.float32)
    wk = np.asarray(inputs["wk"], np.float32)
    wv = np.asarray(inputs["wv"], np.float32)
    wo = np.asarray(inputs["wo"], np.float32)
    psi_g = np.asarray(inputs["psi_g"], np.float32)
    psi_b = np.asarray(inputs["psi_b"], np.float32)
    rc_w = np.asarray(inputs["rc_w"], np.float32)
    rc_b = np.asarray(inputs["rc_b"], np.float32)
    bn2_g = np.asarray(inputs["bn2_g"], np.float32)
    bn2_b = np.asarray(inputs["bn2_b"], np.float32)
    bn2_m = np.asarray(inputs["bn2_m"], np.float32)
    bn2_v = np.asarray(inputs["bn2_v"], np.float32)

    s1 = bn1_g / np.sqrt(bn1_v + BN_EPS)
    mcT = np.ascontiguousarray((mc_w[:, :, 0, 0] * s1[:, None]).T)
    b1 = (mc_b - bn1_m) * s1 + bn1_b
    s2 = bn2_g / np.sqrt(bn2_v + BN_EPS)
    rcT = np.ascontiguousarray((rc_w[:, :, 0, 0] * s2[:, None]).T)  # [o, c]
    b2 = (rc_b - bn2_m) * s2 + bn2_b
    wrc = wo @ rcT                                                  # [s, c]

    # pe_w [d, c, py, px] -> pew[k = cb*64+pp][c128, d]
    pew = np.ascontiguousarray(
        pe_w.transpose(1, 2, 3, 0).reshape(4, 128, 64, DEC).transpose(0, 2, 1, 3)
    ).reshape(256, 128, DEC)
    qw = pew.reshape(-1, DEC) @ wq                   # [256*128, 512]
    qw = qw.reshape(16, 16, 128, DEC).transpose(0, 2, 1, 3) \
        .reshape(16, 128, 16 * DEC)
    import ml_dtypes
    qw = (qw * 32.0).astype(ml_dtypes.float8_e3m4)
    qb = pe_b @ wq                                   # [512]

    c16 = np.zeros((128, C16_W), np.float32)
    c16[:, WK_O:WK_O + 6 * SKIP] = \
        wk.reshape(6, 128, SKIP).transpose(1, 0, 2).reshape(128, -1)
    c16[:, WV_O:WV_O + 6 * SKIP] = \
        wv.reshape(6, 128, SKIP).transpose(1, 0, 2).reshape(128, -1)
    c16[:, MCT_O:MCT_O + 4 * SKIP] = \
        mcT.reshape(4, 128, SKIP).transpose(1, 0, 2).reshape(128, -1)
    c16[:, WRC_O:WRC_O + 4 * SKIP] = \
        wrc.reshape(4, 128, SKIP).transpose(1, 0, 2).reshape(128, -1)
    c16[:, ID_O:ID_O + 128] = np.eye(128, dtype=np.float32)

    c32 = np.zeros((128, C32_W), np.float32)
    c32[:, 0:4] = b1.reshape(4, 128).T
    c32[:, 4:8] = b2.reshape(4, 128).T
    c32[:, 8:12] = qb.reshape(4, 128).T
    c32[:, 12] = psi_g[0]
    c32[:, 13] = psi_b[0]

    shared = {
        "qw": qw,
        "c32": c32,
    }
    in_maps = []
    for c in range(N_CORES):
        m = dict(shared)
        cc = c16.copy()
        cc[:, TRT_O:TRT_O + 6 * NPAT] = \
            trans[c].T.reshape(6, 128, NPAT).transpose(1, 0, 2).reshape(128, -1)
        m["c16"] = cc.astype(f16)
        m["dec"] = np.ascontiguousarray(
            decoder[c].reshape(CIN, NPR, P, NPR, P).transpose(0, 2, 4, 1, 3)
            .reshape(CIN, 64 * NPAT)).astype(f16)
        in_maps.append(m)
    return in_maps


_NC_CACHE: dict = {}


def get_nc(repeat: int = 1):
    if repeat not in _NC_CACHE:
        _NC_CACHE[repeat] = build_nc(repeat)
    return _NC_CACHE[repeat]


def kernel(**inputs) -> np.ndarray:
    nc = get_nc(1)
    in_maps = prepare_in_maps(inputs)
    res = run_bass_kernel_spmd(nc, in_maps, core_ids=list(range(N_CORES)))
    outs = []
    for c in range(N_CORES):
        oq = res.results[c]["out"].reshape(SKIP, P, P, NPR, NPR)
        outs.append(oq.transpose(0, 3, 1, 4, 2).reshape(SKIP, IMG, IMG))
    return np.stack(outs).astype(np.float32)


if __name__ == "__main__":
    nc = build_nc(1)
    print("build ok")
